# revision 25
# baseline (speedup 1.0000x reference)
"""Trainium2 Bass kernel: 5-head MHA + post-softmax A_ds weighting + fc
+ ELU adapter + residual + LayerNorm  (nn_MultiHeadAttention_89472758710361).

Sharding: data-parallel over batch — 16 batches -> 2 per core x 8 cores.
All inputs replicated except input_Q/K/V (batch-sharded). No collectives.

All matmuls run in float32r (full fp32 data at 1 cycle/row on the TRN2 PE
for moving free dim >= 256).

Per-core dataflow (per batch b):
  V    = Xv @ Wv              natural [L, 5*dv] layout   (lhsT = XvT chunks)
  QT,KT: [dk, L] per head     (lhsT = W chunks, rhs = XT chunks)
  per head h:
    S^T[k,q]   = KT[:,kblk].T @ QT            (PSUM, 2x N=512)
    E = exp(S^T * 1/sqrt(dk))                 (ACT, PSUM->SBUF)
    denom[1,q] += ones[128,1].T @ E           (PE partition-reduction)
    EA = E * A^T[kblk]                        (DVE)
    ctxT[dv,q] += V[kblk,hdv].T @ EA          (PSUM accumulate)
    rec = 1/denom (DVE); bcast to [128,q] via PE ones-row matmul
    ctxT_norm = ctxT * bcast(rec)             (DVE, PSUM->SBUF)
  hid = elu(ctx @ (Wfc@a1_w) + a1_b)          (W1' precomputed host-side)
  hidT via PE transpose; ones row appended for the a2 bias
  per q-block: one PSUM accumulation =
        ctx @ Wfc  +  I.T @ residual  +  hidT.T @ [a2_w; a2_b]
  LayerNorm stats via ACT accum_out (sum) + DVE tensor_tensor_reduce (sumsq),
  rstd = exp(-0.5*ln(var+eps)), apply on ACT, DMA out.
"""

import math

import numpy as np

import concourse.bacc as bacc
import concourse.bass as bass
import concourse.tile as tile
from concourse import mybir
from concourse.bass_utils import run_bass_kernel_spmd

F32 = mybir.dt.float32
F32R = mybir.dt.float32r
AF = mybir.ActivationFunctionType
ALU = mybir.AluOpType

B, L, D = 16, 1024, 640
H, DK, DV = 5, 128, 128
NCORES = 8
BPC = B // NCORES  # batches per core
NDC = D // 128     # 5 d_model chunks
NQB = L // 128     # 8 q blocks
NKB = L // 128     # 8 k blocks
SCALE = 1.0 / math.sqrt(DK)
LN_EPS = 1e-5

# ---------------------------------------------------------------- config
USE_F32R = True          # float32r matmuls (full-rate fp32)
BCAST_VIA_DMA = False    # broadcast 1/denom across partitions via DMA (else PE)


def _r(ap):
    """Matmul operand tiles are natively float32r now."""
    return ap


def build_module() -> bass.Bass:
    nc = bacc.Bacc("TRN2", target_bir_lowering=False)

    # ---- DRAM I/O (per-core shard shapes) ----
    xqT_d = nc.dram_tensor("xqT", [BPC, NDC, 128, L], F32R, kind="ExternalInput")[:]
    xkT_d = nc.dram_tensor("xkT", [BPC, NDC, 128, L], F32R, kind="ExternalInput")[:]
    xvT_d = nc.dram_tensor("xvT", [BPC, NDC, 128, L], F32R, kind="ExternalInput")[:]
    res_d = nc.dram_tensor("resid", [BPC, NQB, 128, D], F32, kind="ExternalInput")[:]
    at_d = nc.dram_tensor("at", [NKB, 128, L], F32, kind="ExternalInput")[:]
    wq_d = nc.dram_tensor("wq", [NDC, 128, D], F32R, kind="ExternalInput")[:]
    wk_d = nc.dram_tensor("wk", [NDC, 128, D], F32R, kind="ExternalInput")[:]
    wv_d = nc.dram_tensor("wv", [NDC, 128, D], F32R, kind="ExternalInput")[:]
    wfc_d = nc.dram_tensor("wfc", [H, 128, D], F32R, kind="ExternalInput")[:]
    w1p_d = nc.dram_tensor("w1p", [H, 128, 8], F32R, kind="ExternalInput")[:]
    a1b_d = nc.dram_tensor("a1b", [128, 8], F32, kind="ExternalInput")[:]
    a2s_d = nc.dram_tensor("a2s", [9, D], F32R, kind="ExternalInput")[:]
    ones_d = nc.dram_tensor("ones", [128, 1], F32R, kind="ExternalInput")[:]
    onesr_d = nc.dram_tensor("onesr", [1, 128], F32R, kind="ExternalInput")[:]
    id_d = nc.dram_tensor("ident", [128, 128], F32, kind="ExternalInput")[:]
    out_d = nc.dram_tensor("out", [BPC, L, D], F32, kind="ExternalOutput")[:]

    with tile.TileContext(nc) as tc:
        with (
            tc.tile_pool(name="consts", bufs=1) as cp,
            tc.tile_pool(name="wp", bufs=10) as wp,
            tc.tile_pool(name="xp", bufs=10) as xp,
            tc.tile_pool(name="big", bufs=1) as bp,
            tc.tile_pool(name="work", bufs=2) as kp,
            tc.tile_pool(name="ps", bufs=2, space="PSUM") as ps,
        ):
            # ---- constants ----
            at_sb = cp.tile([128, NKB, L], F32, name="at_sb")
            w1p_sb = cp.tile([128, H, 8], F32R, name="w1p_sb")
            nc.sync.dma_start(out=w1p_sb, in_=w1p_d.rearrange("c p n -> p c n"))
            a1b_sb = cp.tile([128, 8], F32, name="a1b_sb")
            nc.sync.dma_start(out=a1b_sb, in_=a1b_d)
            a2s_sb = cp.tile([9, D], F32R, name="a2s_sb")
            nc.sync.dma_start(out=a2s_sb, in_=a2s_d)
            ones_sb = cp.tile([128, 1], F32R, name="ones_sb")
            nc.sync.dma_start(out=ones_sb, in_=ones_d)
            onesr_sb = cp.tile([1, 128], F32R, name="onesr_sb")
            nc.sync.dma_start(out=onesr_sb, in_=onesr_d)
            id_sb = cp.tile([128, 128], F32, name="id_sb")
            nc.sync.dma_start(out=id_sb, in_=id_d)
            eps_sb = cp.tile([128, 1], F32, name="eps_sb")
            nc.vector.memset(eps_sb, LN_EPS)

            for b in range(BPC):
                # ================= V projection: V[L, 5*dv] natural =================
                xv, wv = [], []
                for c in range(NDC):
                    t = wp.tile([128, D], F32R, tag="wch", name=f"wv{b}_{c}")
                    nc.sync.dma_start(out=t, in_=wv_d[c])
                    wv.append(t)
                    t = xp.tile([128, L], F32R, tag="xch", name=f"xv{b}_{c}")
                    nc.sync.dma_start(out=t, in_=xvT_d[b, c])
                    xv.append(t)
                v_sb = bp.tile([128, NKB, D], F32R, tag="V", name=f"v_sb{b}")
                for lc in range(NKB):
                    # [128, 2, 512] so each 320-wide matmul output stays in one
                    # PSUM bank (a matmul output cannot cross a bank boundary)
                    vps = ps.tile([128, 2, 512], F32, tag="big", name=f"vps{b}_{lc}")
                    for c in range(NDC):
                        for i in range(2):
                            nc.tensor.matmul(
                                vps[:, i, 0:320],
                                lhsT=_r(xv[c][:, lc * 128 : (lc + 1) * 128]),
                                rhs=_r(wv[c][:, i * 320 : i * 320 + 320]),
                                start=(c == 0),
                                stop=(c == NDC - 1),
                                skip_group_check=True,
                            )
                    nc.scalar.copy(
                        out=v_sb[:, lc, :].rearrange("p (a b) -> p a b", a=2),
                        in_=vps[:, :, 0:320],
                    )

                # ====== prefetch Xq/Xk chunks + Wq/Wk (per-head projections) =======
                xq, xk, wqs, wks = [], [], [], []
                for c in range(NDC):
                    t = xp.tile([128, L], F32R, tag="xch", name=f"xq{b}_{c}")
                    nc.sync.dma_start(out=t, in_=xqT_d[b, c])
                    xq.append(t)
                    t = wp.tile([128, D], F32R, tag="wch", name=f"wq{b}_{c}")
                    nc.sync.dma_start(out=t, in_=wq_d[c])
                    wqs.append(t)
                for c in range(NDC):
                    t = xp.tile([128, L], F32R, tag="xch", name=f"xk{b}_{c}")
                    nc.sync.dma_start(out=t, in_=xkT_d[b, c])
                    xk.append(t)
                    t = wp.tile([128, D], F32R, tag="wch", name=f"wk{b}_{c}")
                    nc.sync.dma_start(out=t, in_=wk_d[c])
                    wks.append(t)

                if b == 0:
                    # A^T split into per-k-block DMAs so the first heads'
                    # E*A multiplies wait only on their own 512KB chunk
                    for kb in range(NKB):
                        nc.sync.dma_start(out=at_sb[:, kb, :], in_=at_d[kb])

                def project_head(xs, ws, h, tagn):
                    """One head's [dk, L] projection: PSUM accumulate + copy."""
                    dst = kp.tile([128, L], F32R, tag=tagn, name=f"{tagn}{b}_{h}")
                    pp = ps.tile([128, L], F32, tag="big", name=f"pp{tagn}{b}_{h}")
                    for qs in (0, 512):
                        for c in range(NDC):
                            nc.tensor.matmul(
                                pp[:, qs : qs + 512],
                                lhsT=_r(ws[c][:, h * 128 : (h + 1) * 128]),
                                rhs=_r(xs[c][:, qs : qs + 512]),
                                start=(c == 0),
                                stop=(c == NDC - 1),
                                skip_group_check=True,
                            )
                    nc.scalar.copy(out=dst, in_=pp)
                    return dst

                # ================= attention per head ==============================
                ctxT = bp.tile([128, H, L], F32R, tag="CTX", name=f"ctxT{b}")
                qt_n = project_head(xq, wqs, 0, "qth")
                kt_n = project_head(xk, wks, 0, "kth")
                for h in range(H):
                    qt, kt = qt_n, kt_n
                    ctx_ps = ps.tile([128, L], F32, tag="ctx", bufs=1,
                                     name=f"ctx{b}_{h}")
                    dn = [
                        ps.tile([1, 512], F32, tag="small", name=f"dn{b}_{h}_{i}")
                        for i in range(2)
                    ]

                    # software-pipelined emission: keep the PE stream 2
                    # k-blocks ahead on the S^T matmuls so PE never stalls
                    # waiting for ACT-exp / DVE results of the current block
                    st_t = {}

                    def emit_st(kb, h=h):
                        st = ps.tile([128, L], F32, tag="big",
                                     name=f"st{b}_{h}_{kb}")
                        for qs in (0, 512):
                            nc.tensor.matmul(
                                st[:, qs : qs + 512],
                                lhsT=_r(kt[:, kb * 128 : (kb + 1) * 128]),
                                rhs=_r(qt[:, qs : qs + 512]),
                                start=True,
                                stop=True,
                                skip_group_check=True,
                            )
                        st_t[kb] = st

                    emit_st(0)
                    emit_st(1)
                    for kb in range(NKB):
                        st = st_t.pop(kb)
                        es = kp.tile([128, L], F32R, tag="es", bufs=3,
                                     name=f"es{b}_{h}_{kb}")
                        nc.scalar.activation(out=es, in_=st, func=AF.Exp, scale=SCALE)
                        if kb + 2 < NKB:
                            emit_st(kb + 2)
                        if kb == 1 and h + 1 < H:
                            qt_n = project_head(xq, wqs, h + 1, "qth")
                            kt_n = project_head(xk, wks, h + 1, "kth")
                        for i, qs in enumerate((0, 512)):
                            nc.tensor.matmul(
                                dn[i],
                                lhsT=_r(ones_sb),
                                rhs=_r(es[:, qs : qs + 512]),
                                start=(kb == 0),
                                stop=(kb == NKB - 1),
                                skip_group_check=True,
                            )
                        ea = kp.tile([128, L], F32R, tag="ea", bufs=3,
                                     name=f"ea{b}_{h}_{kb}")
                        nc.vector.tensor_mul(ea, es, at_sb[:, kb, :])
                        for qs in (0, 512):
                            nc.tensor.matmul(
                                ctx_ps[:, qs : qs + 512],
                                lhsT=_r(v_sb[:, kb, h * 128 : (h + 1) * 128]),
                                rhs=_r(ea[:, qs : qs + 512]),
                                start=(kb == 0),
                                stop=(kb == NKB - 1),
                                skip_group_check=True,
                            )
                    # softmax denominator -> reciprocal -> broadcast across parts
                    rec = kp.tile([1, L], F32R, tag="es", bufs=3, name=f"rec{b}_{h}")
                    with nc.allow_low_precision(
                        reason="rec feeds the f32r broadcast matmul; tf32 "
                        "rounding of 1/denom is ~5e-4 relative, acceptable"
                    ):
                        for i, qs in enumerate((0, 512)):
                            nc.vector.reciprocal(out=rec[:, qs : qs + 512], in_=dn[i])
                    bc = kp.tile([128, L], F32, tag="ea", bufs=3, name=f"bc{b}_{h}")
                    if BCAST_VIA_DMA:
                        src = bass.AP(
                            tensor=rec.tensor,
                            offset=rec.offset,
                            ap=[[0, 128]] + list(rec.ap[1:]),
                        )
                        nc.sync.dma_start(out=bc, in_=src)
                    else:
                        bc_ps = ps.tile([128, L], F32, tag="big",
                                        name=f"bcp{b}_{h}")
                        for qs in (0, 512):
                            nc.tensor.matmul(
                                bc_ps[:, qs : qs + 512],
                                lhsT=_r(onesr_sb),
                                rhs=_r(rec[:, qs : qs + 512]),
                                start=True,
                                stop=True,
                                skip_group_check=True,
                            )
                        nc.scalar.copy(out=bc, in_=bc_ps)
                    nc.vector.tensor_mul(ctxT[:, h, :], ctx_ps, bc)

                # ================= adapter hidden: hid = elu(ctx @ W1' + a1b) ======
                hidz = kp.tile([128, NQB, 8], F32, tag="hidz", bufs=1, name=f"hidz{b}")
                for qb in range(NQB):
                    hp = ps.tile([128, 8], F32, tag="small", name=f"hp{b}_{qb}")
                    for h in range(H):
                        nc.tensor.matmul(
                            hp,
                            lhsT=_r(ctxT[:, h, qb * 128 : (qb + 1) * 128]),
                            rhs=_r(w1p_sb[:, h, :]),
                            start=(h == 0),
                            stop=(h == H - 1),
                            skip_group_check=True,
                        )
                    nc.vector.tensor_add(hidz[:, qb, :], hp, a1b_sb)
                hz = hidz.rearrange("p a b -> p (a b)")
                he = kp.tile([128, NQB * 8], F32, tag="he", bufs=1, name=f"he{b}")
                te = kp.tile([128, NQB * 8], F32, tag="te", bufs=1, name=f"te{b}")
                nc.scalar.activation(out=te, in_=hz, func=AF.Exp)
                nc.vector.tensor_scalar(
                    out=te, in0=te, scalar1=1.0, scalar2=0.0,
                    op0=ALU.subtract, op1=ALU.min,
                )
                nc.vector.tensor_scalar(
                    out=he, in0=hz, scalar1=0.0, scalar2=None, op0=ALU.max,
                )
                nc.vector.tensor_add(he, he, te)
                he3 = he.rearrange("p (a b) -> p a b", a=NQB)
                # transpose to hidT [9, qb, 128] with a ones row for the a2 bias
                hidT = kp.tile([9, NQB, 128], F32R, tag="hidT", bufs=1,
                               name=f"hidT{b}")
                nc.vector.memset(hidT.bitcast(F32), 1.0)  # row 8 stays 1.0 (a2 bias row)
                for qb in range(NQB):
                    tp = ps.tile([8, 128], F32, tag="small", name=f"tp{b}_{qb}")
                    nc.tensor.transpose(tp, he3[:, qb, :], id_sb)
                    nc.vector.tensor_copy(hidT[0:8, qb, :], tp)

                # ================= fc + residual + adapter-out + LayerNorm =========
                wfc = []
                for h in range(H):
                    t = wp.tile([128, D], F32R, tag="wch", name=f"wfc{b}_{h}")
                    nc.sync.dma_start(out=t, in_=wfc_d[h])
                    wfc.append(t)
                mv_a = kp.tile([128, NQB, 2], F32, tag="stats", bufs=1,
                               name=f"mv{b}")
                xsb_l = []
                for qb in range(NQB):
                    rt = kp.tile([128, D], F32, tag="resid", bufs=2,
                                 name=f"rt{b}_{qb}")
                    nc.sync.dma_start(out=rt, in_=res_d[b, qb])
                    xps = ps.tile([128, 2, 512], F32, tag="big", name=f"xps{b}_{qb}")
                    for i in range(2):
                        n0 = i * 320
                        for h in range(H):
                            nc.tensor.matmul(
                                xps[:, i, 0:320],
                                lhsT=_r(ctxT[:, h, qb * 128 : (qb + 1) * 128]),
                                rhs=_r(wfc[h][:, n0 : n0 + 320]),
                                start=(h == 0),
                                stop=False,
                                skip_group_check=True,
                            )
                        nc.tensor.matmul(
                            xps[:, i, 0:320],
                            lhsT=_r(hidT[:, qb, :]),
                            rhs=_r(a2s_sb[:, n0 : n0 + 320]),
                            start=False,
                            stop=True,
                            skip_group_check=True,
                        )
                    xsb = kp.tile([128, D], F32, tag="xsb", bufs=5,
                                  name=f"xsb{b}_{qb}")
                    nc.vector.tensor_add(
                        xsb.rearrange("p (a b) -> p a b", a=2),
                        xps[:, :, 0:320],
                        rt.rearrange("p (a b) -> p a b", a=2),
                    )
                    bst = kp.tile([128, 2, 6], F32, tag="bst", name=f"bst{b}_{qb}")
                    for i in range(2):
                        nc.vector.bn_stats(
                            out=bst[:, i, :], in_=xsb[:, i * 320 : (i + 1) * 320])
                    nc.vector.bn_aggr(out=mv_a[:, qb, :], in_=bst)
                    xsb_l.append(xsb)
                    # LayerNorm scale/shift per half-batch (4 q-blocks):
                    # shortens the end-of-kernel tail vs a full-batch barrier
                    if qb % 4 == 3:
                        g = qb - 3
                        sl = slice(g, qb + 1)
                        mean = mv_a[:, sl, 0]
                        rstd = kp.tile([128, 4], F32, tag="rstd", name=f"rs{b}_{g}")
                        nc.scalar.activation(out=rstd, in_=mv_a[:, sl, 1],
                                             func=AF.Ln, bias=eps_sb)
                        nc.scalar.activation(out=rstd, in_=rstd, func=AF.Exp,
                                             scale=-0.5)
                        nmr = kp.tile([128, 4], F32, tag="nmr", name=f"nm{b}_{g}")
                        nc.vector.tensor_mul(nmr, mean, rstd)
                        nc.vector.tensor_scalar(
                            out=nmr, in0=nmr, scalar1=-1.0, scalar2=None,
                            op0=ALU.mult,
                        )
                        for j in range(4):
                            nc.scalar.activation(
                                out=xsb_l[g + j], in_=xsb_l[g + j], func=AF.Identity,
                                bias=nmr[:, j : j + 1], scale=rstd[:, j : j + 1],
                            )
                            nc.sync.dma_start(
                                out=out_d[b, (g + j) * 128 : (g + j + 1) * 128, :],
                                in_=xsb_l[g + j])
    nc.compile()
    return nc


_NC_CACHE = None


def _get_module():
    global _NC_CACHE
    if _NC_CACHE is None:
        _NC_CACHE = build_module()
    return _NC_CACHE


def _tf32(x: np.ndarray) -> np.ndarray:
    """Round fp32 to tf32 (10-bit mantissa) with round-to-nearest-even.

    float32r matmul operands are processed at tf32 precision by the PE;
    pre-rounding on the host makes the data match what the HW computes on
    and keeps the CoreSim numerics aligned with the device."""
    u = np.ascontiguousarray(x, dtype=np.float32).view(np.uint32)
    r = (u + 0x0FFF + ((u >> 13) & 1)) & np.uint32(0xFFFFE000)
    return r.view(np.float32)


def make_in_maps(inputs: dict) -> list[dict]:
    f = lambda x: np.ascontiguousarray(np.asarray(x, dtype=np.float32))
    iq, ik, iv = f(inputs["input_Q"]), f(inputs["input_K"]), f(inputs["input_V"])
    a_ds = f(inputs["A_ds"])
    wq, wk, wv, wfc = f(inputs["Wq"]), f(inputs["Wk"]), f(inputs["Wv"]), f(inputs["Wfc"])
    a1w, a1b = f(inputs["a1_w"]), f(inputs["a1_b"])
    a2w, a2b = f(inputs["a2_w"]), f(inputs["a2_b"])

    shared = {
        "at": np.ascontiguousarray(a_ds.T).reshape(NKB, 128, L),
        "wq": wq.reshape(NDC, 128, D),
        "wk": wk.reshape(NDC, 128, D),
        "wv": wv.reshape(NDC, 128, D),
        "wfc": wfc.reshape(H, 128, D),
        "w1p": (wfc.astype(np.float64) @ a1w.astype(np.float64))
        .astype(np.float32).reshape(H, 128, 8),
        "a1b": np.tile(a1b.reshape(1, 8), (128, 1)),
        "a2s": np.concatenate([a2w, a2b.reshape(1, D)], axis=0),
        "ones": np.ones((128, 1), np.float32),
        "onesr": np.ones((1, 128), np.float32),
        "ident": np.eye(128, dtype=np.float32),
    }
    shared = {k: np.ascontiguousarray(v.astype(np.float32)) for k, v in shared.items()}
    for k in ("wq", "wk", "wv", "wfc", "w1p", "a2s", "ones", "onesr"):
        shared[k] = _tf32(shared[k])

    in_maps = []
    for c in range(NCORES):
        sl = slice(c * BPC, (c + 1) * BPC)
        m = dict(shared)
        m["xqT"] = _tf32(np.ascontiguousarray(
            iq[sl].transpose(0, 2, 1)).reshape(BPC, NDC, 128, L))
        m["xkT"] = _tf32(np.ascontiguousarray(
            ik[sl].transpose(0, 2, 1)).reshape(BPC, NDC, 128, L))
        m["xvT"] = _tf32(np.ascontiguousarray(
            iv[sl].transpose(0, 2, 1)).reshape(BPC, NDC, 128, L))
        m["resid"] = np.ascontiguousarray(iq[sl]).reshape(BPC, NQB, 128, D)
        in_maps.append(m)
    return in_maps


def kernel(**inputs) -> np.ndarray:
    nc = _get_module()
    in_maps = make_in_maps(inputs)
    res = run_bass_kernel_spmd(nc, in_maps, core_ids=list(range(NCORES)))
    return np.concatenate([r["out"] for r in res.results], axis=0)


# revision 26
# speedup vs baseline: 95.9186x; 95.9186x over previous
"""Trainium2 Bass kernel: 5-head MHA + post-softmax A_ds weighting + fc
+ ELU adapter + residual + LayerNorm  (nn_MultiHeadAttention_89472758710361).

Sharding: data-parallel over batch — 16 batches -> 2 per core x 8 cores.
All inputs replicated except input_Q/K/V (batch-sharded). No collectives.

All matmuls run in float32r (full fp32 data at 1 cycle/row on the TRN2 PE
for moving free dim >= 256).

Per-core dataflow (per batch b):
  V    = Xv @ Wv              natural [L, 5*dv] layout   (lhsT = XvT chunks)
  QT,KT: [dk, L] per head     (lhsT = W chunks, rhs = XT chunks)
  per head h:
    S^T[k,q]   = KT[:,kblk].T @ QT            (PSUM, 2x N=512)
    E = exp(S^T * 1/sqrt(dk))                 (ACT, PSUM->SBUF)
    denom[1,q] += ones[128,1].T @ E           (PE partition-reduction)
    EA = E * A^T[kblk]                        (DVE)
    ctxT[dv,q] += V[kblk,hdv].T @ EA          (PSUM accumulate)
    rec = 1/denom (DVE); bcast to [128,q] via PE ones-row matmul
    ctxT_norm = ctxT * bcast(rec)             (DVE, PSUM->SBUF)
  hid = elu(ctx @ (Wfc@a1_w) + a1_b)          (W1' precomputed host-side)
  hidT via PE transpose; ones row appended for the a2 bias
  per q-block: one PSUM accumulation =
        ctx @ Wfc  +  I.T @ residual  +  hidT.T @ [a2_w; a2_b]
  LayerNorm stats via ACT accum_out (sum) + DVE tensor_tensor_reduce (sumsq),
  rstd = exp(-0.5*ln(var+eps)), apply on ACT, DMA out.
"""

import math

import numpy as np

import concourse.bacc as bacc
import concourse.bass as bass
import concourse.tile as tile
from concourse import mybir
from concourse.bass_utils import run_bass_kernel_spmd

F32 = mybir.dt.float32
F32R = mybir.dt.float32r
AF = mybir.ActivationFunctionType
ALU = mybir.AluOpType

B, L, D = 16, 1024, 640
H, DK, DV = 5, 128, 128
NCORES = 8
BPC = B // NCORES  # batches per core
NDC = D // 128     # 5 d_model chunks
NQB = L // 128     # 8 q blocks
NKB = L // 128     # 8 k blocks
SCALE = 1.0 / math.sqrt(DK)
LN_EPS = 1e-5

# ---------------------------------------------------------------- config
USE_F32R = True          # float32r matmuls (full-rate fp32)
BCAST_VIA_DMA = False    # broadcast 1/denom across partitions via DMA (else PE)


def _r(ap):
    """Matmul operand tiles are natively float32r now."""
    return ap


def build_module() -> bass.Bass:
    nc = bacc.Bacc("TRN2", target_bir_lowering=False)

    # ---- DRAM I/O (per-core shard shapes) ----
    xqT_d = nc.dram_tensor("xqT", [BPC, NDC, 128, L], F32R, kind="ExternalInput")[:]
    xkT_d = nc.dram_tensor("xkT", [BPC, NDC, 128, L], F32R, kind="ExternalInput")[:]
    xvT_d = nc.dram_tensor("xvT", [BPC, NDC, 128, L], F32R, kind="ExternalInput")[:]
    res_d = nc.dram_tensor("resid", [BPC, NQB, 128, D], F32, kind="ExternalInput")[:]
    at_d = nc.dram_tensor("at", [NKB, 128, L], F32, kind="ExternalInput")[:]
    wq_d = nc.dram_tensor("wq", [NDC, 128, D], F32R, kind="ExternalInput")[:]
    wk_d = nc.dram_tensor("wk", [NDC, 128, D], F32R, kind="ExternalInput")[:]
    wv_d = nc.dram_tensor("wv", [NDC, 128, D], F32R, kind="ExternalInput")[:]
    wfc_d = nc.dram_tensor("wfc", [H, 128, D], F32R, kind="ExternalInput")[:]
    w1p_d = nc.dram_tensor("w1p", [H, 128, 8], F32R, kind="ExternalInput")[:]
    a1b_d = nc.dram_tensor("a1b", [128, 8], F32, kind="ExternalInput")[:]
    a2s_d = nc.dram_tensor("a2s", [9, D], F32R, kind="ExternalInput")[:]
    ones_d = nc.dram_tensor("ones", [128, 1], F32R, kind="ExternalInput")[:]
    onesr_d = nc.dram_tensor("onesr", [1, 128], F32R, kind="ExternalInput")[:]
    id_d = nc.dram_tensor("ident", [128, 128], F32, kind="ExternalInput")[:]
    out_d = nc.dram_tensor("out", [BPC, L, D], F32, kind="ExternalOutput")[:]

    with tile.TileContext(nc) as tc:
        with (
            tc.tile_pool(name="consts", bufs=1) as cp,
            tc.tile_pool(name="wp", bufs=10) as wp,
            tc.tile_pool(name="xp", bufs=10) as xp,
            tc.tile_pool(name="big", bufs=1) as bp,
            tc.tile_pool(name="work", bufs=2) as kp,
            tc.tile_pool(name="ps", bufs=2, space="PSUM") as ps,
        ):
            # ---- constants ----
            at_sb = cp.tile([128, NKB, L], F32, name="at_sb")
            w1p_sb = cp.tile([128, H, 8], F32R, name="w1p_sb")
            nc.sync.dma_start(out=w1p_sb, in_=w1p_d.rearrange("c p n -> p c n"))
            a1b_sb = cp.tile([128, 8], F32, name="a1b_sb")
            nc.sync.dma_start(out=a1b_sb, in_=a1b_d)
            a2s_sb = cp.tile([9, D], F32R, name="a2s_sb")
            nc.sync.dma_start(out=a2s_sb, in_=a2s_d)
            ones_sb = cp.tile([128, 1], F32R, name="ones_sb")
            nc.sync.dma_start(out=ones_sb, in_=ones_d)
            onesr_sb = cp.tile([1, 128], F32R, name="onesr_sb")
            nc.sync.dma_start(out=onesr_sb, in_=onesr_d)
            id_sb = cp.tile([128, 128], F32, name="id_sb")
            nc.sync.dma_start(out=id_sb, in_=id_d)
            eps_sb = cp.tile([128, 1], F32, name="eps_sb")
            nc.vector.memset(eps_sb, LN_EPS)

            for b in range(BPC):
                # ================= V projection: V[L, 5*dv] natural =================
                xv, wv = [], []
                for c in range(NDC):
                    t = wp.tile([128, D], F32R, tag="wch", name=f"wv{b}_{c}")
                    nc.sync.dma_start(out=t, in_=wv_d[c])
                    wv.append(t)
                    t = xp.tile([128, L], F32R, tag="xch", name=f"xv{b}_{c}")
                    nc.sync.dma_start(out=t, in_=xvT_d[b, c])
                    xv.append(t)
                v_sb = bp.tile([128, NKB, D], F32R, tag="V", name=f"v_sb{b}")
                for lc in range(NKB):
                    # [128, 2, 512] so each 320-wide matmul output stays in one
                    # PSUM bank (a matmul output cannot cross a bank boundary)
                    vps = ps.tile([128, 2, 512], F32, tag="big", name=f"vps{b}_{lc}")
                    for c in range(NDC):
                        for i in range(2):
                            nc.tensor.matmul(
                                vps[:, i, 0:320],
                                lhsT=_r(xv[c][:, lc * 128 : (lc + 1) * 128]),
                                rhs=_r(wv[c][:, i * 320 : i * 320 + 320]),
                                start=(c == 0),
                                stop=(c == NDC - 1),
                                skip_group_check=True,
                            )
                    nc.scalar.copy(
                        out=v_sb[:, lc, :].rearrange("p (a b) -> p a b", a=2),
                        in_=vps[:, :, 0:320],
                    )

                # ====== prefetch Xq/Xk chunks + Wq/Wk (per-head projections) =======
                xq, xk, wqs, wks = [], [], [], []
                for c in range(NDC):
                    t = xp.tile([128, L], F32R, tag="xch", name=f"xq{b}_{c}")
                    nc.sync.dma_start(out=t, in_=xqT_d[b, c])
                    xq.append(t)
                    t = wp.tile([128, D], F32R, tag="wch", name=f"wq{b}_{c}")
                    nc.sync.dma_start(out=t, in_=wq_d[c])
                    wqs.append(t)
                for c in range(NDC):
                    t = xp.tile([128, L], F32R, tag="xch", name=f"xk{b}_{c}")
                    nc.sync.dma_start(out=t, in_=xkT_d[b, c])
                    xk.append(t)
                    t = wp.tile([128, D], F32R, tag="wch", name=f"wk{b}_{c}")
                    nc.sync.dma_start(out=t, in_=wk_d[c])
                    wks.append(t)

                if b == 0:
                    # A^T split into per-k-block DMAs so the first heads'
                    # E*A multiplies wait only on their own 512KB chunk
                    for kb in range(NKB):
                        nc.sync.dma_start(out=at_sb[:, kb, :], in_=at_d[kb])

                def project_head(xs, ws, h, tagn):
                    """One head's [dk, L] projection: PSUM accumulate + copy."""
                    dst = kp.tile([128, L], F32R, tag=tagn, name=f"{tagn}{b}_{h}")
                    pp = ps.tile([128, L], F32, tag="big", name=f"pp{tagn}{b}_{h}")
                    for qs in (0, 512):
                        for c in range(NDC):
                            nc.tensor.matmul(
                                pp[:, qs : qs + 512],
                                lhsT=_r(ws[c][:, h * 128 : (h + 1) * 128]),
                                rhs=_r(xs[c][:, qs : qs + 512]),
                                start=(c == 0),
                                stop=(c == NDC - 1),
                                skip_group_check=True,
                            )
                    nc.scalar.copy(out=dst, in_=pp)
                    return dst

                # ================= attention per head ==============================
                ctxT = bp.tile([128, H, L], F32R, tag="CTX", name=f"ctxT{b}")
                qt_n = project_head(xq, wqs, 0, "qth")
                kt_n = project_head(xk, wks, 0, "kth")
                for h in range(H):
                    qt, kt = qt_n, kt_n
                    ctx_ps = ps.tile([128, L], F32, tag="ctx", bufs=1,
                                     name=f"ctx{b}_{h}")
                    dn = [
                        ps.tile([1, 512], F32, tag="small", name=f"dn{b}_{h}_{i}")
                        for i in range(2)
                    ]

                    # software-pipelined emission: keep the PE stream 2
                    # k-blocks ahead on the S^T matmuls so PE never stalls
                    # waiting for ACT-exp / DVE results of the current block
                    st_t = {}

                    def emit_st(kb, h=h):
                        st = ps.tile([128, L], F32, tag="big",
                                     name=f"st{b}_{h}_{kb}")
                        for qs in (0, 512):
                            nc.tensor.matmul(
                                st[:, qs : qs + 512],
                                lhsT=_r(kt[:, kb * 128 : (kb + 1) * 128]),
                                rhs=_r(qt[:, qs : qs + 512]),
                                start=True,
                                stop=True,
                                skip_group_check=True,
                            )
                        st_t[kb] = st

                    emit_st(0)
                    emit_st(1)
                    for kb in range(NKB):
                        st = st_t.pop(kb)
                        es = kp.tile([128, L], F32R, tag="es", bufs=3,
                                     name=f"es{b}_{h}_{kb}")
                        nc.scalar.activation(out=es, in_=st, func=AF.Exp, scale=SCALE)
                        if kb + 2 < NKB:
                            emit_st(kb + 2)
                        if kb == 1 and h + 1 < H:
                            qt_n = project_head(xq, wqs, h + 1, "qth")
                            kt_n = project_head(xk, wks, h + 1, "kth")
                        for i, qs in enumerate((0, 512)):
                            nc.tensor.matmul(
                                dn[i],
                                lhsT=_r(ones_sb),
                                rhs=_r(es[:, qs : qs + 512]),
                                start=(kb == 0),
                                stop=(kb == NKB - 1),
                                skip_group_check=True,
                            )
                        ea = kp.tile([128, L], F32R, tag="ea", bufs=3,
                                     name=f"ea{b}_{h}_{kb}")
                        nc.vector.tensor_mul(ea, es, at_sb[:, kb, :])
                        for qs in (0, 512):
                            nc.tensor.matmul(
                                ctx_ps[:, qs : qs + 512],
                                lhsT=_r(v_sb[:, kb, h * 128 : (h + 1) * 128]),
                                rhs=_r(ea[:, qs : qs + 512]),
                                start=(kb == 0),
                                stop=(kb == NKB - 1),
                                skip_group_check=True,
                            )
                    # softmax denominator -> reciprocal -> broadcast across parts
                    rec = kp.tile([1, L], F32R, tag="es", bufs=3, name=f"rec{b}_{h}")
                    with nc.allow_low_precision(
                        reason="rec feeds the f32r broadcast matmul; tf32 "
                        "rounding of 1/denom is ~5e-4 relative, acceptable"
                    ):
                        for i, qs in enumerate((0, 512)):
                            nc.vector.reciprocal(out=rec[:, qs : qs + 512], in_=dn[i])
                    bc = kp.tile([128, L], F32, tag="ea", bufs=3, name=f"bc{b}_{h}")
                    if BCAST_VIA_DMA:
                        src = bass.AP(
                            tensor=rec.tensor,
                            offset=rec.offset,
                            ap=[[0, 128]] + list(rec.ap[1:]),
                        )
                        nc.sync.dma_start(out=bc, in_=src)
                    else:
                        bc_ps = ps.tile([128, L], F32, tag="big",
                                        name=f"bcp{b}_{h}")
                        for qs in (0, 512):
                            nc.tensor.matmul(
                                bc_ps[:, qs : qs + 512],
                                lhsT=_r(onesr_sb),
                                rhs=_r(rec[:, qs : qs + 512]),
                                start=True,
                                stop=True,
                                skip_group_check=True,
                            )
                        nc.scalar.copy(out=bc, in_=bc_ps)
                    nc.vector.tensor_mul(ctxT[:, h, :], ctx_ps, bc)

                # ================= adapter hidden: hid = elu(ctx @ W1' + a1b) ======
                hidz = kp.tile([128, NQB, 8], F32, tag="hidz", bufs=1, name=f"hidz{b}")
                for qb in range(NQB):
                    hp = ps.tile([128, 8], F32, tag="small", name=f"hp{b}_{qb}")
                    for h in range(H):
                        nc.tensor.matmul(
                            hp,
                            lhsT=_r(ctxT[:, h, qb * 128 : (qb + 1) * 128]),
                            rhs=_r(w1p_sb[:, h, :]),
                            start=(h == 0),
                            stop=(h == H - 1),
                            skip_group_check=True,
                        )
                    nc.vector.tensor_add(hidz[:, qb, :], hp, a1b_sb)
                hz = hidz.rearrange("p a b -> p (a b)")
                he = kp.tile([128, NQB * 8], F32, tag="he", bufs=1, name=f"he{b}")
                te = kp.tile([128, NQB * 8], F32, tag="te", bufs=1, name=f"te{b}")
                nc.scalar.activation(out=te, in_=hz, func=AF.Exp)
                nc.vector.tensor_scalar(
                    out=te, in0=te, scalar1=1.0, scalar2=0.0,
                    op0=ALU.subtract, op1=ALU.min,
                )
                nc.vector.tensor_scalar(
                    out=he, in0=hz, scalar1=0.0, scalar2=None, op0=ALU.max,
                )
                nc.vector.tensor_add(he, he, te)
                he3 = he.rearrange("p (a b) -> p a b", a=NQB)
                # transpose to hidT [9, qb, 128] with a ones row for the a2 bias
                hidT = kp.tile([9, NQB, 128], F32R, tag="hidT", bufs=1,
                               name=f"hidT{b}")
                nc.vector.memset(hidT.bitcast(F32), 1.0)  # row 8 stays 1.0 (a2 bias row)
                for qb in range(NQB):
                    tp = ps.tile([8, 128], F32, tag="small", name=f"tp{b}_{qb}")
                    nc.tensor.transpose(tp, he3[:, qb, :], id_sb)
                    nc.vector.tensor_copy(hidT[0:8, qb, :], tp)

                # ================= fc + residual + adapter-out + LayerNorm =========
                wfc = []
                for h in range(H):
                    t = wp.tile([128, D], F32R, tag="wch", name=f"wfc{b}_{h}")
                    nc.sync.dma_start(out=t, in_=wfc_d[h])
                    wfc.append(t)
                mv_a = kp.tile([128, NQB, 2], F32, tag="stats", bufs=1,
                               name=f"mv{b}")
                xsb_l = []
                for qb in range(NQB):
                    rt = kp.tile([128, D], F32, tag="resid", bufs=2,
                                 name=f"rt{b}_{qb}")
                    nc.sync.dma_start(out=rt, in_=res_d[b, qb])
                    xps = ps.tile([128, 2, 512], F32, tag="big", name=f"xps{b}_{qb}")
                    for i in range(2):
                        n0 = i * 320
                        for h in range(H):
                            nc.tensor.matmul(
                                xps[:, i, 0:320],
                                lhsT=_r(ctxT[:, h, qb * 128 : (qb + 1) * 128]),
                                rhs=_r(wfc[h][:, n0 : n0 + 320]),
                                start=(h == 0),
                                stop=False,
                                skip_group_check=True,
                            )
                        nc.tensor.matmul(
                            xps[:, i, 0:320],
                            lhsT=_r(hidT[:, qb, :]),
                            rhs=_r(a2s_sb[:, n0 : n0 + 320]),
                            start=False,
                            stop=True,
                            skip_group_check=True,
                        )
                    xsb = kp.tile([128, D], F32, tag="xsb", bufs=5,
                                  name=f"xsb{b}_{qb}")
                    nc.vector.tensor_add(
                        xsb.rearrange("p (a b) -> p a b", a=2),
                        xps[:, :, 0:320],
                        rt.rearrange("p (a b) -> p a b", a=2),
                    )
                    bst = kp.tile([128, 2, 6], F32, tag="bst", name=f"bst{b}_{qb}")
                    for i in range(2):
                        nc.vector.bn_stats(
                            out=bst[:, i, :], in_=xsb[:, i * 320 : (i + 1) * 320])
                    nc.vector.bn_aggr(out=mv_a[:, qb, :], in_=bst)
                    xsb_l.append(xsb)
                    # LayerNorm scale/shift per half-batch (4 q-blocks):
                    # shortens the end-of-kernel tail vs a full-batch barrier
                    if qb % 4 == 3:
                        g = qb - 3
                        sl = slice(g, qb + 1)
                        mean = mv_a[:, sl, 0]
                        rstd = kp.tile([128, 4], F32, tag="rstd", name=f"rs{b}_{g}")
                        nc.scalar.activation(out=rstd, in_=mv_a[:, sl, 1],
                                             func=AF.Ln, bias=eps_sb)
                        nc.scalar.activation(out=rstd, in_=rstd, func=AF.Exp,
                                             scale=-0.5)
                        nmr = kp.tile([128, 4], F32, tag="nmr", name=f"nm{b}_{g}")
                        nc.vector.tensor_mul(nmr, mean, rstd)
                        nc.vector.tensor_scalar(
                            out=nmr, in0=nmr, scalar1=-1.0, scalar2=None,
                            op0=ALU.mult,
                        )
                        for j in range(4):
                            nc.scalar.activation(
                                out=xsb_l[g + j], in_=xsb_l[g + j], func=AF.Identity,
                                bias=nmr[:, j : j + 1], scale=rstd[:, j : j + 1],
                            )
                            nc.sync.dma_start(
                                out=out_d[b, (g + j) * 128 : (g + j + 1) * 128, :],
                                in_=xsb_l[g + j])
    nc.compile()
    return nc


_NC_CACHE = None


def _get_module():
    global _NC_CACHE
    if _NC_CACHE is None:
        _NC_CACHE = build_module()
    return _NC_CACHE


def _tf32(x: np.ndarray) -> np.ndarray:
    """Round fp32 to tf32 (10-bit mantissa) with round-to-nearest-even.

    float32r matmul operands are processed at tf32 precision by the PE;
    pre-rounding on the host makes the data match what the HW computes on
    and keeps the CoreSim numerics aligned with the device."""
    u = np.ascontiguousarray(x, dtype=np.float32).view(np.uint32)
    r = (u + 0x0FFF + ((u >> 13) & 1)) & np.uint32(0xFFFFE000)
    return r.view(np.float32)


def make_in_maps(inputs: dict) -> list[dict]:
    f = lambda x: np.ascontiguousarray(np.asarray(x, dtype=np.float32))
    iq, ik, iv = f(inputs["input_Q"]), f(inputs["input_K"]), f(inputs["input_V"])
    a_ds = f(inputs["A_ds"])
    wq, wk, wv, wfc = f(inputs["Wq"]), f(inputs["Wk"]), f(inputs["Wv"]), f(inputs["Wfc"])
    a1w, a1b = f(inputs["a1_w"]), f(inputs["a1_b"])
    a2w, a2b = f(inputs["a2_w"]), f(inputs["a2_b"])

    shared = {
        "at": np.ascontiguousarray(a_ds.T).reshape(NKB, 128, L),
        "wq": wq.reshape(NDC, 128, D),
        "wk": wk.reshape(NDC, 128, D),
        "wv": wv.reshape(NDC, 128, D),
        "wfc": wfc.reshape(H, 128, D),
        "w1p": (wfc.astype(np.float64) @ a1w.astype(np.float64))
        .astype(np.float32).reshape(H, 128, 8),
        "a1b": np.tile(a1b.reshape(1, 8), (128, 1)),
        "a2s": np.concatenate([a2w, a2b.reshape(1, D)], axis=0),
        "ones": np.ones((128, 1), np.float32),
        "onesr": np.ones((1, 128), np.float32),
        "ident": np.eye(128, dtype=np.float32),
    }
    shared = {k: np.ascontiguousarray(v.astype(np.float32)) for k, v in shared.items()}
    for k in ("wq", "wk", "wv", "wfc", "w1p", "a2s", "ones", "onesr"):
        shared[k] = _tf32(shared[k])

    in_maps = []
    for c in range(NCORES):
        sl = slice(c * BPC, (c + 1) * BPC)
        m = dict(shared)
        m["xqT"] = _tf32(np.ascontiguousarray(
            iq[sl].transpose(0, 2, 1)).reshape(BPC, NDC, 128, L))
        m["xkT"] = _tf32(np.ascontiguousarray(
            ik[sl].transpose(0, 2, 1)).reshape(BPC, NDC, 128, L))
        m["xvT"] = _tf32(np.ascontiguousarray(
            iv[sl].transpose(0, 2, 1)).reshape(BPC, NDC, 128, L))
        m["resid"] = np.ascontiguousarray(iq[sl]).reshape(BPC, NQB, 128, D)
        in_maps.append(m)
    return in_maps


_JIT_CACHE = None


def _get_jitted():
    """Build the 8-core shard_map executable once per process.

    run_bass_kernel_spmd re-traces jax on every call (~250ms); caching the
    jitted callable makes repeat kernel() calls cheap."""
    global _JIT_CACHE
    if _JIT_CACHE is not None:
        return _JIT_CACHE
    import jax
    from jax.sharding import Mesh, PartitionSpec
    from jax.experimental.shard_map import shard_map
    from concourse import mybir
    from concourse.bass2jax import (
        _bass_exec_p, install_neuronx_cc_hook, partition_id_tensor)

    nc = _get_module()
    install_neuronx_cc_hook()
    pname = nc.partition_id_tensor.name if nc.partition_id_tensor else None
    in_names, out_names, out_avals, zero_shapes = [], [], [], []
    for alloc in nc.m.functions[0].allocations:
        if not isinstance(alloc, mybir.MemoryLocationSet):
            continue
        name = alloc.memorylocations[0].name
        if alloc.kind == "ExternalInput":
            if name != pname:
                in_names.append(name)
        elif alloc.kind == "ExternalOutput":
            shape = tuple(alloc.tensor_shape)
            dtype = mybir.dt.np(alloc.dtype)
            out_names.append(name)
            out_avals.append(jax.core.ShapedArray(shape, dtype))
            zero_shapes.append((shape, dtype))
    all_in = list(in_names) + list(out_names)
    if pname is not None:
        all_in.append(pname)

    def _body(*args):
        operands = list(args)
        if pname is not None:
            operands.append(partition_id_tensor())
        return tuple(_bass_exec_p.bind(
            *operands, out_avals=tuple(out_avals), in_names=tuple(all_in),
            out_names=tuple(out_names), lowering_input_output_aliases=(),
            sim_require_finite=True, sim_require_nnan=True, nc=nc))

    devices = jax.devices()[:NCORES]
    mesh = Mesh(np.asarray(devices), ("core",))
    n = len(in_names) + len(out_names)
    sharded = jax.jit(
        shard_map(_body, mesh=mesh, in_specs=(PartitionSpec("core"),) * n,
                  out_specs=(PartitionSpec("core"),) * len(out_names),
                  check_rep=False),
        keep_unused=True,
    )
    _JIT_CACHE = (sharded, in_names, zero_shapes)
    return _JIT_CACHE


def kernel(**inputs) -> np.ndarray:
    in_maps = make_in_maps(inputs)
    try:
        sharded, in_names, zero_shapes = _get_jitted()
        concat_in = [
            np.concatenate([np.asarray(in_maps[c][nm]) for c in range(NCORES)],
                           axis=0)
            for nm in in_names
        ]
        concat_zeros = [
            np.zeros((NCORES * s[0], *s[1:]), d) for s, d in zero_shapes
        ]
        outs = sharded(*concat_in, *concat_zeros)
        return np.asarray(outs[0]).reshape(B, L, D)
    except Exception:
        nc = _get_module()
        res = run_bass_kernel_spmd(nc, in_maps, core_ids=list(range(NCORES)))
        return np.concatenate([r["out"] for r in res.results], axis=0)
